# revision 5
# baseline (speedup 1.0000x reference)
import sys, os
sys.path.insert(0, "/opt/trn_rl_repo")
import numpy as np
import ml_dtypes

from concourse import bacc, tile, mybir, bass_utils

N = 100000
E = 1600000
IN_C, HID_C, OUT_C = 128, 128, 64
EPS = 1e-5
NCORES = 8
NP = N // NCORES            # 12500 dst nodes per core
WIN = 512                   # psum window (nodes)
NWIN = (NP + WIN - 1) // WIN
NCHUNK = 4
CHUNK = N // NCHUNK         # 25000 rows per gather chunk (< int16 range)
BF16 = ml_dtypes.bfloat16

LAST_RESULTS = None         # test.py reads exec_time_ns from here


def _prep(edge_index):
    """Host-side graph prep: degree-balanced node->core map, per-core
    window/chunk edge lists (uniform across cores), idx + S streams."""
    src = np.concatenate([edge_index[0].astype(np.int64), np.arange(N, dtype=np.int64)])
    dst = np.concatenate([edge_index[1].astype(np.int64), np.arange(N, dtype=np.int64)])
    deg = np.bincount(dst, minlength=N).astype(np.float32)
    dinv = 1.0 / np.sqrt(np.maximum(deg, 1.0))
    w = (dinv[src] * dinv[dst]).astype(np.float32)

    # degree-sorted round-robin deal -> nearly identical degree profiles/core
    order = np.argsort(-deg, kind="stable")
    vpos = np.empty(N, dtype=np.int64)
    ranks = np.arange(N, dtype=np.int64)
    vpos[order] = (ranks % NCORES) * NP + ranks // NCORES
    inv_perm = np.empty(N, dtype=np.int64)   # virtual row -> node id
    inv_perm[vpos] = np.arange(N, dtype=np.int64)

    vsrc = vpos[src]
    vdst = vpos[dst]
    core_e = vdst // NP
    dstrel = vdst - core_e * NP
    chunk_e = vsrc // CHUNK
    idxv = (vsrc % CHUNK).astype(np.int32)

    # bucket edges per (core, window, chunk), dst-sorted
    key = ((core_e * NWIN + np.minimum(dstrel // WIN, NWIN - 1)) * NCHUNK + chunk_e)
    sort_o = np.lexsort((dstrel, key))
    key_s = key[sort_o]
    idx_s = idxv[sort_o]
    w_s = w[sort_o]
    drel_s = dstrel[sort_o]
    bounds = np.searchsorted(key_s, np.arange(NCORES * NWIN * NCHUNK + 1))

    # uniform counts per (window, chunk) across cores, pad to mult of 128
    cnt = (bounds[1:] - bounds[:-1]).reshape(NCORES, NWIN, NCHUNK)
    ucnt = cnt.max(axis=0)
    ucnt = ((ucnt + 127) // 128) * 128
    ntile = ucnt // 128                          # [NWIN, NCHUNK]

    # build per-core idx lists and per (w,c,t) column extents
    per_core_idx = []   # list over cores of int16 arrays, concatenated (w,c)
    lo_all = np.full((NWIN, NCHUNK, ntile.max() if ntile.size else 1), 1 << 30, np.int64)
    hi_all = np.full_like(lo_all, -1)
    per_core_cols = []  # per core: list of (w,c)-> per-edge col array (padded -1)
    for c in range(NCORES):
        idx_parts, col_parts = [], []
        for wd in range(NWIN):
            for ch in range(NCHUNK):
                k = (c * NWIN + wd) * NCHUNK + ch
                a, b = bounds[k], bounds[k + 1]
                n = ucnt[wd, ch]
                ii = np.zeros(n, np.int16)
                cc = np.full(n, -1, np.int64)
                ii[: b - a] = idx_s[a:b].astype(np.int16)
                cc[: b - a] = drel_s[a:b] - wd * WIN
                idx_parts.append(ii)
                col_parts.append(cc)
                nt = ntile[wd, ch]
                for t in range(nt):
                    seg = cc[t * 128:(t + 1) * 128]
                    seg = seg[seg >= 0]
                    if seg.size:
                        lo_all[wd, ch, t] = min(lo_all[wd, ch, t], seg.min())
                        hi_all[wd, ch, t] = max(hi_all[wd, ch, t], seg.max())
        per_core_idx.append(idx_parts)
        per_core_cols.append(col_parts)

    # uniform tile spans (rounded), S column offsets
    tile_meta = []   # (wd, ch, t, lo, span, soff)
    soff = 0
    for wd in range(NWIN):
        wsz = min(WIN, NP - wd * WIN)
        for ch in range(NCHUNK):
            for t in range(ntile[wd, ch]):
                lo = lo_all[wd, ch, t]
                hi = hi_all[wd, ch, t]
                if hi < 0:
                    lo, hi = 0, 0
                lo = (lo // 8) * 8
                span = min(((hi - lo) // 8 + 1) * 8, wsz - lo)
                tile_meta.append((wd, ch, t, int(lo), int(span), soff))
                soff += span
    SCOLS = soff

    # per-core S stream + per-core w values aligned to padded lists
    w_pad = np.zeros((NCORES, sum(int(u) for u in ucnt.reshape(-1))), np.float32)
    off = 0
    wc_off = {}
    for wd in range(NWIN):
        for ch in range(NCHUNK):
            wc_off[(wd, ch)] = off
            off += int(ucnt[wd, ch])
    for c in range(NCORES):
        for wd in range(NWIN):
            for ch in range(NCHUNK):
                k = (c * NWIN + wd) * NCHUNK + ch
                a, b = bounds[k], bounds[k + 1]
                o = wc_off[(wd, ch)]
                w_pad[c, o:o + (b - a)] = w_s[a:b]

    S = np.zeros((NCORES, 128, SCOLS), np.float32)
    for (wd, ch, t, lo, span, so) in tile_meta:
        o = wc_off[(wd, ch)] + t * 128
        for c in range(NCORES):
            cols = per_core_cols[c][wd * NCHUNK + ch][t * 128:(t + 1) * 128]
            valid = cols >= 0
            p = np.nonzero(valid)[0]
            if p.size:
                rel = cols[p] - lo
                S[c, p, so + rel] = w_pad[c, o + p]

    # idx stream in dma_gather SBUF layout: [128, total_cols] int16
    IXC = sum(int(u) for u in ucnt.reshape(-1)) // 16
    IX = np.zeros((NCORES, 128, IXC), np.int16)
    ix_col_off = {}
    col = 0
    for wd in range(NWIN):
        for ch in range(NCHUNK):
            ix_col_off[(wd, ch)] = col
            n = int(ucnt[wd, ch])
            for c in range(NCORES):
                arr = per_core_idx[c][wd * NCHUNK + ch]
                B = arr.reshape(n // 16, 16).T          # [16, n/16]
                IX[c, :, col:col + n // 16] = np.tile(B, (8, 1))
            col += n // 16

    meta = dict(ucnt=ucnt, ntile=ntile, tile_meta=tile_meta, wc_off=wc_off,
                ix_col_off=ix_col_off, SCOLS=SCOLS, IXC=IXC)
    return vpos, inv_perm, IX, S, meta


def _build(meta):
    ucnt, ntile, tile_meta = meta["ucnt"], meta["ntile"], meta["tile_meta"]
    wc_off, ix_col_off, SCOLS, IXC = (meta["wc_off"], meta["ix_col_off"],
                                      meta["SCOLS"], meta["IXC"])
    tm_by_wc = {}
    for (wd, ch, t, lo, span, so) in tile_meta:
        tm_by_wc.setdefault((wd, ch), []).append((t, lo, span, so))

    nc = bacc.Bacc(None, target_bir_lowering=False, num_devices=NCORES)
    bf = mybir.dt.bfloat16
    f32 = mybir.dt.float32
    xt = nc.dram_tensor("xt", [N, IN_C], bf, kind="ExternalInput")
    ix = nc.dram_tensor("ix", [128, IXC], mybir.dt.int16, kind="ExternalInput")
    sw = nc.dram_tensor("sw", [128, SCOLS], bf, kind="ExternalInput")
    W1 = nc.dram_tensor("W1", [IN_C, HID_C], f32, kind="ExternalInput")
    W2 = nc.dram_tensor("W2", [HID_C, HID_C], f32, kind="ExternalInput")
    W3 = nc.dram_tensor("W3", [HID_C, OUT_C], f32, kind="ExternalInput")
    bn = nc.dram_tensor("bn", [128, 8], f32, kind="ExternalInput")
    # bn cols: b1,g1,be1,b2,g2,be2,b3(64 rows),eye? -> b3 in col 6
    ident = nc.dram_tensor("ident", [128, 128], f32, kind="ExternalInput")
    h_out = nc.dram_tensor("h_out", [NP, HID_C], f32, kind="ExternalOutput")
    o_out = nc.dram_tensor("o_out", [NP, OUT_C], f32, kind="ExternalOutput")

    h1t = nc.dram_tensor("h1t", [N, HID_C], bf, kind="Internal", addr_space="Shared")
    h2t = nc.dram_tensor("h2t", [N, HID_C], bf, kind="Internal", addr_space="Shared")
    bnc1 = nc.dram_tensor("bnc1", [NP, HID_C], bf, kind="Internal")
    bnc2 = nc.dram_tensor("bnc2", [NP, HID_C], bf, kind="Internal")
    st_in = nc.dram_tensor("st_in", [128, 2], f32, kind="Internal")
    st_out = nc.dram_tensor("st_out", [128 * NCORES, 2], f32, kind="Internal",
                            addr_space="Shared")

    GMAX = int(ntile.max())

    with tile.TileContext(nc) as tc:
        with tc.tile_pool(name="const", bufs=1) as cpool, \
             tc.tile_pool(name="zbuf", bufs=1) as zpool, \
             tc.tile_pool(name="gath", bufs=2) as gpool, \
             tc.tile_pool(name="sbuf", bufs=2) as spool, \
             tc.tile_pool(name="small", bufs=3) as mpool, \
             tc.tile_pool(name="psA", bufs=2, space="PSUM") as psA, \
             tc.tile_pool(name="psB", bufs=2, space="PSUM") as psB, \
             tc.tile_pool(name="psT", bufs=2, space="PSUM") as psT:

            wsb = cpool.tile([128, IN_C + HID_C + OUT_C], f32, tag="w")
            nc.sync.dma_start(wsb[:, 0:HID_C], W1[:])
            nc.sync.dma_start(wsb[:, HID_C:2 * HID_C], W2[:])
            nc.sync.dma_start(wsb[:, 2 * HID_C:2 * HID_C + OUT_C], W3[:])
            bnp = cpool.tile([128, 8], f32, tag="bn")
            nc.sync.dma_start(bnp[:], bn[:])
            idn = cpool.tile([128, 128], f32, tag="id")
            nc.sync.dma_start(idn[:], ident[:])
            szero = cpool.tile([128, WIN], bf, tag="sz")
            nc.vector.memset(szero[:], 0.0)
            z_all = zpool.tile([128, NP], f32, tag="z")

            for layer in range(3):
                tbl = (xt, h1t, h2t)[layer]
                Wap = wsb[:, layer * HID_C: layer * HID_C +
                          (HID_C if layer < 2 else OUT_C)]
                CO = HID_C if layer < 2 else OUT_C
                bias_ap = bnp[:CO, 3 * layer:3 * layer + 1] if layer < 2 \
                    else bnp[:CO, 6:7]
                ssum = mpool.tile([128, NWIN], f32, tag=f"ss{layer}")
                ssq = mpool.tile([128, NWIN], f32, tag=f"sq{layer}")

                for wd in range(NWIN):
                    wsz = min(WIN, NP - wd * WIN)
                    # idx + S slices for this window
                    c0 = ix_col_off[(wd, 0)]
                    c1 = (ix_col_off[(wd + 1, 0)] if wd + 1 < NWIN else IXC)
                    ixt = spool.tile([128, c1 - c0], mybir.dt.int16, tag="ix")
                    nc.sync.dma_start(ixt[:], ix[:, c0:c1])
                    s0 = tm_by_wc[(wd, 0)][0][3]
                    last = tm_by_wc[(wd, NCHUNK - 1)][-1]
                    s1 = last[3] + last[2]
                    st = spool.tile([128, s1 - s0], bf, tag="sw")
                    nc.sync.dma_start(st[:], sw[:, s0:s1])

                    gts = []
                    for ch in range(NCHUNK):
                        n = int(ucnt[wd, ch])
                        gt = gpool.tile([128, GMAX, 128], bf, tag=f"g{ch}")
                        rel = ix_col_off[(wd, ch)] - c0
                        nc.gpsimd.dma_gather(
                            gt[:, :n // 128, :],
                            tbl[ch * CHUNK:(ch + 1) * CHUNK, :],
                            ixt[:, rel:rel + n // 16],
                            num_idxs=n, num_idxs_reg=n, elem_size=128,
                            single_packet=False)
                        gts.append(gt)

                    apsum = psA.tile([128, WIN], f32, tag="agg")
                    nc.tensor.matmul(apsum[:, :wsz], gts[0][:, 0, :],
                                     szero[:, :wsz], start=True, stop=False,
                                     skip_group_check=True)
                    nmm = sum(len(tm_by_wc[(wd, ch)]) for ch in range(NCHUNK))
                    k = 0
                    for ch in range(NCHUNK):
                        for (t, lo, span, so) in tm_by_wc[(wd, ch)]:
                            k += 1
                            nc.tensor.matmul(
                                apsum[:, lo:lo + span], gts[ch][:, t, :],
                                st[:, so - s0:so - s0 + span],
                                start=False, stop=(k == nmm),
                                skip_group_check=True)
                    agg = mpool.tile([128, WIN], f32, tag="agg_sb")
                    nc.vector.tensor_copy(agg[:, :wsz], apsum[:, :wsz])
                    zps = psB.tile([128, WIN], f32, tag="zp")
                    nc.tensor.matmul(zps[:CO, :wsz], Wap, agg[:, :wsz],
                                     start=True, stop=True,
                                     skip_group_check=True)
                    zslice = z_all[:CO, wd * WIN: wd * WIN + wsz]
                    nc.scalar.activation(zslice, zps[:CO, :wsz],
                                         mybir.ActivationFunctionType.Identity,
                                         bias=bias_ap,
                                         accum_out=ssum[:CO, wd:wd + 1])
                    if layer < 2:
                        scr = mpool.tile([128, WIN], f32, tag="scr")
                        nc.scalar.activation(scr[:CO, :wsz], zslice,
                                             mybir.ActivationFunctionType.Square,
                                             accum_out=ssq[:CO, wd:wd + 1])

                if layer < 2:
                    # global BN stats via AllGather of per-core partials
                    part = mpool.tile([128, 2], f32, tag="part")
                    nc.vector.tensor_reduce(part[:, 0:1], ssum[:],
                                            mybir.AxisListType.X,
                                            mybir.AluOpType.add)
                    nc.vector.tensor_reduce(part[:, 1:2], ssq[:],
                                            mybir.AxisListType.X,
                                            mybir.AluOpType.add)
                    nc.sync.dma_start(st_in[:], part[:])
                    nc.gpsimd.collective_compute(
                        "AllGather", mybir.AluOpType.bypass,
                        replica_groups=[list(range(NCORES))],
                        ins=[st_in[:].opt()], outs=[st_out[:].opt()])
                    gl = mpool.tile([128, NCORES * 2], f32, tag="gl")
                    glv = gl[:].rearrange("p (a b) -> p a b", a=NCORES)
                    nc.sync.dma_start(
                        glv, st_out[:].rearrange("(a p) b -> p a b", p=128))
                    mean = mpool.tile([128, 4], f32, tag="mv")
                    nc.vector.tensor_reduce(
                        mean[:, 0:1], glv[:, :, 0:1],
                        mybir.AxisListType.XY, mybir.AluOpType.add)
                    nc.vector.tensor_reduce(
                        mean[:, 1:2], glv[:, :, 1:2],
                        mybir.AxisListType.XY, mybir.AluOpType.add)
                    nc.vector.tensor_scalar_mul(mean[:, 0:2], mean[:, 0:2], 1.0 / N)
                    # var = E[x^2] - mean^2 ; invstd
                    nc.vector.tensor_tensor(mean[:, 2:3], mean[:, 0:1],
                                            mean[:, 0:1], mybir.AluOpType.mult)
                    nc.vector.tensor_tensor(mean[:, 2:3], mean[:, 1:2],
                                            mean[:, 2:3], mybir.AluOpType.subtract)
                    nc.vector.tensor_scalar_add(mean[:, 2:3], mean[:, 2:3], EPS)
                    nc.scalar.sqrt(mean[:, 3:4], mean[:, 2:3])
                    inv = mpool.tile([128, 3], f32, tag="inv")
                    nc.vector.reciprocal(inv[:, 0:1], mean[:, 3:4])
                    gam = bnp[:, 3 * layer + 1:3 * layer + 2]
                    bet = bnp[:, 3 * layer + 2:3 * layer + 3]
                    nc.vector.tensor_tensor(inv[:, 1:2], gam, inv[:, 0:1],
                                            mybir.AluOpType.mult)   # gamma'
                    nc.vector.tensor_tensor(inv[:, 2:3], inv[:, 1:2],
                                            mean[:, 0:1], mybir.AluOpType.mult)
                    nc.vector.tensor_tensor(inv[:, 2:3], bet, inv[:, 2:3],
                                            mybir.AluOpType.subtract)  # beta'
                    nc.scalar.activation(z_all[:], z_all[:],
                                         mybir.ActivationFunctionType.Relu,
                                         bias=inv[:, 2:3], scale=inv[:, 1:2])

                # transpose to row-major + emit
                bnc = (bnc1, bnc2, None)[layer]
                NB = (NP + 127) // 128
                for kb in range(NB):
                    r0 = kb * 128
                    rn = min(128, NP - r0)
                    tp = psT.tile([128, 128], f32, tag="tp")
                    nc.tensor.transpose(tp[:rn, :CO], z_all[:CO, r0:r0 + rn],
                                        idn[:CO, :CO])
                    if layer < 2:
                        stg = mpool.tile([128, 128], bf, tag="stg")
                        nc.vector.tensor_copy(stg[:rn, :CO], tp[:rn, :CO])
                        nc.sync.dma_start(bnc[r0:r0 + rn, :], stg[:rn, :CO])
                    if layer == 1:
                        stf = mpool.tile([128, 128], f32, tag="stf")
                        nc.vector.tensor_copy(stf[:rn, :CO], tp[:rn, :CO])
                        nc.sync.dma_start(h_out[r0:r0 + rn, :], stf[:rn, :CO])
                    if layer == 2:
                        stf = mpool.tile([128, 128], f32, tag="stf")
                        nc.vector.tensor_copy(stf[:rn, :CO], tp[:rn, :CO])
                        nc.sync.dma_start(o_out[r0:r0 + rn, :], stf[:rn, :CO])
                if layer < 2:
                    ht = (h1t, h2t)[layer]
                    nc.gpsimd.collective_compute(
                        "AllGather", mybir.AluOpType.bypass,
                        replica_groups=[list(range(NCORES))],
                        ins=[bnc[:].opt()], outs=[ht[:].opt()])
    nc.compile()
    return nc


def kernel(x, W1, b1, g1, be1, W2, b2, g2, be2, W3, b3, edge_index):
    global LAST_RESULTS
    x = np.asarray(x); edge_index = np.asarray(edge_index)
    vpos, inv_perm, IX, S, meta = _prep(edge_index)

    nc = _build(meta)

    xt = np.zeros((N, IN_C), np.float32)
    xt[vpos] = np.asarray(x, np.float32)
    xt = xt.astype(BF16)
    bn = np.zeros((128, 8), np.float32)
    bn[:, 0] = np.asarray(b1, np.float32)
    bn[:, 1] = np.asarray(g1, np.float32)
    bn[:, 2] = np.asarray(be1, np.float32)
    bn[:, 3] = np.asarray(b2, np.float32)
    bn[:, 4] = np.asarray(g2, np.float32)
    bn[:, 5] = np.asarray(be2, np.float32)
    bn[:64, 6] = np.asarray(b3, np.float32)
    ident = np.eye(128, dtype=np.float32)

    in_maps = []
    for c in range(NCORES):
        in_maps.append({
            "xt": xt, "ix": IX[c], "sw": S[c].astype(BF16),
            "W1": np.asarray(W1, np.float32), "W2": np.asarray(W2, np.float32),
            "W3": np.asarray(W3, np.float32), "bn": bn, "ident": ident,
        })
    os.environ["BASS_NEVER_TRACE"] = "1"   # no NTFF hook in this container
    res = bass_utils.run_bass_kernel_spmd(nc, in_maps, list(range(NCORES)))
    LAST_RESULTS = res
    global _LAST_NC, _LAST_INMAPS
    _LAST_NC, _LAST_INMAPS = nc, in_maps
    h_v = np.concatenate([res.results[c]["h_out"] for c in range(NCORES)], axis=0)
    o_v = np.concatenate([res.results[c]["o_out"] for c in range(NCORES)], axis=0)
    h = h_v[vpos]
    out = o_v[vpos]
    return (h.astype(np.float32), out.astype(np.float32))
